# revision 14
# baseline (speedup 1.0000x reference)
"""CLUES loss (focal CE + supervised contrastive) on 8 Trainium2 NeuronCores.

v2: circulant-triangle sharding.  The N x N sim matrix is covered once per
unordered pair: row-tile a (128 rows) covers col-tiles a..a+32 (mod 64).
Core k owns row-tiles {k, 8+k, ..., 56+k}; the host feeds each core its
row-rotated copy of the data so one static program serves all cores.

Per core: fp8 DoubleRow matmuls produce sim psum chunks; ACT applies
exp(k*psum) in place with accum_out row-sums; ap=1 transposed matmuls on the
PE produce per-column sums (diag tile excluded once).  The host sums the 8
cores' row/col partial-S vectors, takes the log, and assembles the loss --
the O(N^2) and O(N*D) work all happens on device; the host does O(N)
unsharding only.

Normalization: row norms from fp8 bn_stats (split DVE/Pool); rb = 8/||x||
broadcast across partitions via a DRAM round trip; the transposed fp8 copy
(host-provided, ktile-major) is scaled by rb on DVE to give xn; xn feeds the
Gram, the class-sum matmuls (via oh*rb weights), and the per-row class dots
M8 used for the masked term.
"""

import sys

if '/opt/trn_rl_repo' not in sys.path:
    sys.path.insert(0, '/opt/trn_rl_repo')

import numpy as np
import ml_dtypes

import concourse.bass as bass
import concourse.mybir as mybir
import concourse.tile as tile
from concourse.vector_clock import ScopedClock
from concourse.bass_utils import run_bass_kernel_spmd

F32 = mybir.dt.float32
BF16 = mybir.dt.bfloat16
FP8 = mybir.dt.float8e4
I32 = mybir.dt.int32
AF = mybir.ActivationFunctionType
ALU = mybir.AluOpType
AX = mybir.AxisListType
DR = mybir.MatmulPerfMode.DoubleRow

FP8NP = ml_dtypes.float8_e4m3
BF16NP = ml_dtypes.bfloat16

N, C, D = 8192, 8, 256
NCORES = 8
T64 = 64                       # col tiles
MT = 8                         # own row tiles (CE side)
GAMMA = 2.0
LS = 0.1
ALPHA = 0.3
TEMP = 0.07
C1 = 8.0                       # xn = x_hat * C1  ->  sim psum = 64*cos
KAPPA = 1.0 / (C1 * C1 * TEMP)  # exp scale: kappa*psum = cos/T
LN_C1 = float(np.log(C1))

STRIPES = [0, 8, 16, 24, 32, 40, 48, 56]
# chunks per stripe: list of (col_tile0, n_tiles); <=12 tiles per chunk
CHUNKS = {
    0:  [(0, 12), (12, 12), (24, 9)],
    8:  [(8, 12), (20, 12), (32, 9)],
    16: [(16, 12), (28, 12), (40, 9)],
    24: [(24, 12), (36, 12), (48, 9)],
    32: [(32, 12), (44, 12), (56, 8)],
    40: [(40, 12), (52, 12), (0, 8)],
    48: [(48, 12), (60, 4), (0, 12), (12, 4)],
    56: [(56, 8), (0, 12), (12, 12)],
}
NCHMAX = 4

# colsum bookkeeping: col tile -> ordered list of (stripe_idx, chunk_idx,
# slice_within_chunk); diag slice (col==stripe) excluded.
_COLTOUCH = {}
for _si, _a in enumerate(STRIPES):
    for _ci, (_c0, _nt) in enumerate(CHUNKS[_a]):
        for _s in range(_nt):
            _c = _c0 + _s
            if _c == _a:
                continue
            _COLTOUCH.setdefault(_c, []).append((_si, _ci, _s))


# ---------------------------------------------------------------------------
# walrus in this container only accepts ONE semaphore wait per instruction,
# while Tile freely attaches several.  Patch 1 fixes the final drain; patch 2
# is a post-pass hoisting extra waits onto same-engine NoOp carriers.
# ---------------------------------------------------------------------------

def _patched_drain_and_barrier(self, tick_clock, wait_clock):
    nc = self.nc
    carrier = nc.sync.nop(nofuse=True, hint="drain_wait_carrier")
    wait_clock.add_sem_waits(carrier.ins, ScopedClock({None: tick_clock.global_clock}))
    si = carrier.ins.sync_info
    waits = list(si.on_wait or []) if si is not None else []
    if len(waits) > 1:
        carrier.ins.sync_info = mybir.SyncInfo(
            on_wait=waits[:1], on_update=list(si.on_update or []))
        for w in waits[1:]:
            n2 = nc.sync.nop(nofuse=True, hint="drain_wait_carrier")
            n2.ins.sync_info = mybir.SyncInfo(on_wait=[w], on_update=[])
    nc.sync.drain()
    nc.all_engine_barrier()
    popped = nc._tile_sem_poison_stack.pop()
    assert popped is self._sem_poison
    nc.clear_and_free_semaphores(list(self.sems.allocated().values()))
    nc.all_engine_barrier()


tile.TileContext._drain_and_barrier = _patched_drain_and_barrier


def _split_multi_waits(nc):
    """One sem wait per instruction: move extras to NoOp carriers just before."""
    n_split = 0
    for f in nc.m.functions:
        for bb in f.blocks:
            new = []
            for inst in bb.instructions:
                si = inst.sync_info
                waits = list(si.on_wait or []) if si is not None else []
                if len(waits) > 1:
                    for w in waits[:-1]:
                        nop = mybir.InstNoOp(
                            name=f"{inst.name}-wsplit{n_split}",
                            engine=inst.engine,
                            bass_nofuse=True,
                            sync_info=mybir.SyncInfo(on_wait=[w], on_update=[]),
                        )
                        n_split += 1
                        new.append(nop)
                    inst.sync_info = mybir.SyncInfo(
                        on_wait=[waits[-1]], on_update=list(si.on_update or []))
                new.append(inst)
            bb.instructions[:] = new


# ---------------------------------------------------------------------------
# kernel build
# ---------------------------------------------------------------------------

def _build():
    nc = bass.Bass()

    xT_d = nc.dram_tensor("xT", [128, 2, N], FP8, kind="ExternalInput")
    xrm_d = nc.dram_tensor("xrm", [N, D], FP8, kind="ExternalInput")
    tpo_d = nc.dram_tensor("tpo", [N], F32, kind="ExternalInput")   # rotated
    lg_d = nc.dram_tensor("lg", [N // 8, C], F32, kind="ExternalInput")
    to_d = nc.dram_tensor("to", [N // 8], F32, kind="ExternalInput")

    sacc_d = nc.dram_tensor("sacc", [128, 8, NCHMAX], F32, kind="ExternalOutput")
    csum_d = nc.dram_tensor("csum", [128, T64], F32, kind="ExternalOutput")
    m8_d = nc.dram_tensor("m8", [128, T64, C], F32, kind="ExternalOutput")
    cnt_d = nc.dram_tensor("cnt", [C, 1], F32, kind="ExternalOutput")
    focal_d = nc.dram_tensor("focal", [128, MT], F32, kind="ExternalOutput")

    r_scr = nc.dram_tensor("r_scr", [N], BF16, kind="Internal")
    cs_scr = nc.dram_tensor("cs_scr", [C, D], BF16, kind="Internal")

    with tile.TileContext(nc) as tc:
        with (
            tc.tile_pool(name="big", bufs=1) as big,
            tc.tile_pool(name="work", bufs=3) as work,
            tc.tile_pool(name="ebfp", bufs=2) as ebfp,
            tc.tile_pool(name="psb", bufs=2, space="PSUM") as psb,
            tc.tile_pool(name="psc", bufs=1, space="PSUM") as psc,
            tc.tile_pool(name="pss", bufs=1, space="PSUM") as pss,
        ):
            # ============ phase 0: loads + ACT warm =========================
            warm = big.tile([128, 1], F32, tag="warm")
            nc.vector.memset(warm, 0.0)
            nc.scalar.activation(out=warm, in_=warm, func=AF.Exp)

            xT8 = big.tile([128, 2, N], FP8, tag="xT8")
            nc.sync.dma_start(out=xT8, in_=xT_d.ap())
            x8rm = big.tile([128, 64, D], FP8, tag="x8rm")
            nc.sync.dma_start(out=x8rm,
                              in_=xrm_d.ap().rearrange("(t p) d -> p t d", p=128))

            lg = big.tile([128, MT, C], F32, tag="lg")
            nc.scalar.dma_start(out=lg, in_=lg_d.ap().rearrange("(t p) c -> p t c", p=128))
            to_pi = big.tile([128, MT], F32, tag="to_pi")
            nc.scalar.dma_start(out=to_pi, in_=to_d.ap().rearrange("(t p) -> p t", p=128))
            t_po = big.tile([128, T64], F32, tag="t_po")
            nc.scalar.dma_start(out=t_po, in_=tpo_d.ap().rearrange("(t p) -> p t", p=128))

            from concourse.masks import make_identity
            ident16 = big.tile([128, 128], BF16, tag="ident16")
            make_identity(nc, ident16)

            # iotas for one-hots
            iota8_i = big.tile([128, C], I32, tag="iota8i")
            nc.gpsimd.iota(iota8_i, pattern=[[1, C]], base=0, channel_multiplier=0)
            iota8 = big.tile([128, C], F32, tag="iota8")
            nc.vector.tensor_copy(out=iota8, in_=iota8_i)

            # ============ phase 1: focal CE (fills ACT early) ===============
            oh_own = big.tile([128, MT, C], F32, tag="oh_own")
            nc.vector.tensor_tensor(
                out=oh_own,
                in0=to_pi.to_broadcast([128, MT, C]),
                in1=bass.AP(tensor=iota8.tensor, offset=iota8.offset,
                            ap=[iota8.ap[0], [0, MT], iota8.ap[1]]),
                op=ALU.is_equal)
            focal = big.tile([128, MT], F32, tag="focal")
            mx = big.tile([128, MT], F32, tag="mx")
            nc.vector.reduce_max(out=mx, in_=lg, axis=AX.X)
            nmx = big.tile([128, MT], F32, tag="nmx")
            nc.vector.tensor_scalar(out=nmx, in0=mx, scalar1=-1.0, scalar2=None,
                                    op0=ALU.mult, op1=ALU.bypass)
            sumexp = big.tile([128, MT], F32, tag="sumexp")
            for m in range(MT):
                esc = work.tile([128, C], F32, tag="esc")
                nc.scalar.activation(out=esc, in_=lg[:, m, :], func=AF.Exp,
                                     bias=nmx[:, m:m + 1], scale=1.0,
                                     accum_out=sumexp[:, m:m + 1])
            logZ = big.tile([128, MT], F32, tag="logZ")
            nc.scalar.activation(out=logZ, in_=sumexp, func=AF.Ln)
            nc.vector.tensor_tensor(out=logZ, in0=logZ, in1=mx, op=ALU.add)
            xt = big.tile([128, MT], F32, tag="xt")
            ohl = work.tile([128, MT, C], F32, tag="ohl")
            nc.vector.tensor_tensor(out=ohl, in0=lg, in1=oh_own, op=ALU.mult)
            nc.vector.reduce_sum(out=xt, in_=ohl, axis=AX.X)
            sx = big.tile([128, MT], F32, tag="sx")
            nc.vector.reduce_sum(out=sx, in_=lg, axis=AX.X)
            ce = big.tile([128, MT], F32, tag="ce")
            u1 = work.tile([128, MT], F32, tag="u1")
            nc.vector.tensor_scalar(out=u1, in0=xt, scalar1=1.0 - LS, scalar2=None,
                                    op0=ALU.mult, op1=ALU.bypass)
            u2 = work.tile([128, MT], F32, tag="u2")
            nc.vector.tensor_scalar(out=u2, in0=sx, scalar1=LS / C, scalar2=None,
                                    op0=ALU.mult, op1=ALU.bypass)
            nc.vector.tensor_tensor(out=u1, in0=u1, in1=u2, op=ALU.add)
            nc.vector.tensor_tensor(out=ce, in0=logZ, in1=u1, op=ALU.subtract)
            pt_t = work.tile([128, MT], F32, tag="pt")
            nc.scalar.activation(out=pt_t, in_=ce, func=AF.Exp, scale=-1.0)
            nc.vector.tensor_scalar(out=pt_t, in0=pt_t, scalar1=-1.0, scalar2=1.0,
                                    op0=ALU.mult, op1=ALU.add)
            nc.vector.tensor_tensor(out=focal, in0=pt_t, in1=pt_t, op=ALU.mult)
            nc.vector.tensor_tensor(out=focal, in0=focal, in1=ce, op=ALU.mult)
            nc.sync.dma_start(out=focal_d.ap(), in_=focal)

            # one-hot (p-outer) for class sums / counts
            oh_po = big.tile([128, T64, C], BF16, tag="oh_po")
            nc.vector.tensor_tensor(
                out=oh_po,
                in0=t_po.to_broadcast([128, T64, C]),
                in1=bass.AP(tensor=iota8.tensor, offset=iota8.offset,
                            ap=[iota8.ap[0], [0, T64], iota8.ap[1]]),
                op=ALU.is_equal)

            # ============ phase 2: stats -> rb -> xn (pipelined by group) ===
            sumsq = big.tile([128, T64], F32, tag="sumsq")
            lnc1_t = big.tile([128, 1], F32, tag="lnc1")
            nc.vector.memset(lnc1_t, LN_C1)
            rb_po = big.tile([128, T64], BF16, tag="rb_po")
            rb16bc = big.tile([128, N], BF16, tag="rb16bc")
            xn = big.tile([128, 2, N], FP8, tag="xn")
            r_po_ap = r_scr.ap().rearrange("(t p) -> p t", p=128)

            for g in range(8):
                t0 = g * 8
                for t in range(t0, t0 + 8):
                    sqp = work.tile([128, D], BF16, tag="sqp")
                    nc.gpsimd.tensor_tensor(out=sqp, in0=x8rm[:, t, :],
                                            in1=x8rm[:, t, :], op=ALU.mult)
                    nc.vector.reduce_sum(out=sumsq[:, t:t + 1], in_=sqp,
                                         axis=AX.X)
                # rb = C1 / sqrt(sumsq) = exp(-0.5*ln(sumsq) + ln C1)
                lns = work.tile([128, 8], F32, tag="lns")
                nc.scalar.activation(out=lns, in_=sumsq[:, t0:t0 + 8], func=AF.Ln)
                nc.scalar.activation(out=rb_po[:, t0:t0 + 8], in_=lns,
                                     func=AF.Exp, scale=-0.5, bias=lnc1_t[:, 0:1])
                nc.scalar.dma_start(out=r_po_ap[:, t0:t0 + 8],
                                    in_=rb_po[:, t0:t0 + 8])
                # broadcast rb over partitions for cols of this group
                j0 = t0 * 128
                nc.scalar.dma_start(
                    out=rb16bc[:, j0:j0 + 1024],
                    in_=bass.AP(tensor=r_scr.ap().tensor, offset=j0,
                                ap=[[0, 128], [1, 1024]]))
                for kt in range(2):
                    nc.vector.tensor_tensor(out=xn[:, kt, j0:j0 + 1024],
                                            in0=xT8[:, kt, j0:j0 + 1024],
                                            in1=rb16bc[:, j0:j0 + 1024],
                                            op=ALU.mult)

            # ============ phase 3: class sums + counts (PE, off critical) ===
            ohr = big.tile([128, T64, C], FP8, tag="ohr")
            nc.vector.tensor_tensor(
                out=ohr, in0=oh_po,
                in1=bass.AP(tensor=rb_po.tensor, offset=rb_po.offset,
                            ap=[rb_po.ap[0], rb_po.ap[1], [0, C]]),
                op=ALU.mult)
            ones_bf = big.tile([128, 1], BF16, tag="ones_bf")
            nc.vector.memset(ones_bf, 1.0)
            cs_ps = pss.tile([C, D + 1], F32, tag="cs")
            for t in range(T64):
                nc.tensor.matmul(cs_ps[:, 0:D], ohr[:, t, :], x8rm[:, t, :],
                                 start=(t == 0), stop=(t == T64 - 1),
                                 skip_group_check=True)
                nc.tensor.matmul(cs_ps[:, D:D + 1], oh_po[:, t, :], ones_bf,
                                 start=(t == 0), stop=(t == T64 - 1),
                                 skip_group_check=True)
            cnt_sb = big.tile([C, 1], F32, tag="cnt_sb")
            nc.vector.tensor_copy(out=cnt_sb, in_=cs_ps[:, D:D + 1])
            nc.sync.dma_start(out=cnt_d.ap(), in_=cnt_sb)
            cs16 = big.tile([C, D], BF16, tag="cs16")
            nc.vector.tensor_copy(out=cs16, in_=cs_ps[:, 0:D])
            # transpose CS to ktile-major fp8 [128, 2, C] via tiny DRAM trip
            nc.sync.dma_start(out=cs_scr.ap(), in_=cs16)
            cs8t16 = big.tile([128, 2, C], BF16, tag="cs8t16")
            for kt in range(2):
                nc.sync.dma_start(out=cs8t16[:, kt, :], in_=bass.AP(
                    tensor=cs_scr.ap().tensor, offset=kt * 128,
                    ap=[[1, 128], [D, C]]))
            cs8t = big.tile([128, 2, C], FP8, tag="cs8t")
            nc.vector.tensor_copy(out=cs8t, in_=cs8t16)

            # M8[row, c] = xn_row . CS8T  (DoubleRow, 64 tiles)
            # reuses the big-chunk psum pool (runs before the stripe loop)
            m8_full = psb.tile([128, 1536], F32, tag="bigps")
            m8_ps = m8_full[:, 0:T64 * C]
            for t in range(T64):
                nc.tensor.matmul(m8_ps[:, t * C:(t + 1) * C],
                                 xn[:, :, t * 128:(t + 1) * 128], cs8t,
                                 start=True, stop=True, perf_mode=DR,
                                 skip_group_check=True)
            m8_sb = big.tile([128, T64 * C], F32, tag="m8sb")
            nc.vector.tensor_copy(out=m8_sb, in_=m8_ps)
            nc.sync.dma_start(out=m8_d.ap().rearrange("p t c -> p (t c)"),
                              in_=m8_sb)

            # ============ phase 4: stripes: gram + exp + colsums ============
            sacc = big.tile([128, 8, NCHMAX], F32, tag="sacc")
            nc.vector.memset(sacc, 0.0)
            col_ps = psc.tile([128, T64], F32, tag="colps")
            touched = {c: 0 for c in _COLTOUCH}

            for si, a in enumerate(STRIPES):
                lhsT = xn[:, :, a * 128:(a + 1) * 128]
                for ci, (c0, nt) in enumerate(CHUNKS[a]):
                    csz = nt * 128
                    pt = psb.tile([128, 1536], F32, tag="bigps")
                    for n0 in range(0, csz, 512):
                        w = min(512, csz - n0)
                        nc.tensor.matmul(
                            pt[:, n0:n0 + w], lhsT,
                            xn[:, :, (c0 * 128) + n0:(c0 * 128) + n0 + w],
                            start=True, stop=True, perf_mode=DR)
                    ebf = ebfp.tile([128, 1536], BF16, tag="ebf")
                    nc.scalar.activation(out=ebf[:, 0:csz], in_=pt[:, 0:csz],
                                         func=AF.Exp, scale=KAPPA,
                                         accum_out=sacc[:, si, ci:ci + 1])
                    for s in range(nt):
                        c = c0 + s
                        if c == a:
                            continue
                        touched[c] += 1
                        first = touched[c] == 1
                        last = touched[c] == len(_COLTOUCH[c])
                        nc.tensor.matmul(col_ps[:, c:c + 1],
                                         ebf[:, s * 128:(s + 1) * 128], ones_bf,
                                         start=first, stop=last,
                                         skip_group_check=True)

            # ============ phase 5: outputs ==================================
            col_sb = big.tile([128, T64], F32, tag="col_sb")
            nc.vector.tensor_copy(out=col_sb, in_=col_ps)
            nc.sync.dma_start(out=csum_d.ap(), in_=col_sb)
            nc.sync.dma_start(out=sacc_d.ap(), in_=sacc)

    _split_multi_waits(nc)
    return nc


_NC = None
LAST_RESULTS = None
RUN_KWARGS = {}


def _get_nc():
    global _NC
    if _NC is None:
        _NC = _build()
    return _NC


def kernel(logits, embeddings, targets):
    logits = np.ascontiguousarray(np.asarray(logits), dtype=np.float32)
    embeddings = np.ascontiguousarray(np.asarray(embeddings), dtype=np.float32)
    targets_np = np.asarray(targets)
    tf32 = targets_np.astype(np.float32)

    x8 = embeddings.astype(FP8NP)                      # [N, D] fp8
    nc = _get_nc()
    in_maps = []
    for k in range(NCORES):
        roll = np.roll(x8, -k * 128, axis=0)
        xT8 = np.ascontiguousarray(roll.reshape(N, 2, 128).transpose(2, 1, 0))
        sl = slice(k * (N // 8), (k + 1) * (N // 8))
        in_maps.append({
            "xT": xT8,
            "xrm": np.ascontiguousarray(roll),
            "tpo": np.ascontiguousarray(np.roll(tf32, -k * 128)),
            "lg": np.ascontiguousarray(logits[sl]),
            "to": np.ascontiguousarray(tf32[sl]),
        })
    res = run_bass_kernel_spmd(nc, in_maps, core_ids=list(range(NCORES)), **RUN_KWARGS)
    global LAST_RESULTS
    LAST_RESULTS = res

    # ---- host unshard: O(N) assembly --------------------------------------
    S = np.zeros(N, np.float64)
    fsum = 0.0
    for k in range(NCORES):
        r = res.results[k]
        sacc = np.asarray(r["sacc"], np.float64)       # [128, 8, 4]
        for si, a in enumerate(STRIPES):
            b = (a + k) % T64
            S[b * 128:(b + 1) * 128] += sacc[:, si, :].sum(axis=1)
        csum = np.asarray(r["csum"], np.float64)       # [128, 64]
        for c in range(T64):
            b = (c + k) % T64
            S[b * 128:(b + 1) * 128] += csum[:, c]
        fsum += float(np.asarray(r["focal"], np.float64).sum())

    r0 = res.results[0]
    m8 = np.asarray(r0["m8"], np.float64)              # [128, 64, 8] p-inner
    m8r = m8.transpose(1, 0, 2).reshape(N, C)          # row = t*128+p
    cnt = np.asarray(r0["cnt"], np.float64).reshape(C)

    tgt = targets_np.astype(np.int64)
    P = cnt[tgt] - 1.0
    npos = np.maximum(P, 1.0)
    masked = m8r[np.arange(N), tgt] * KAPPA - 1.0 / TEMP
    con_i = (P * np.log(S + 1e-8) - masked) / npos
    con_loss = np.float32(con_i.mean())
    ce_loss = np.float32(fsum / N)
    total = np.float32(ce_loss + np.float32(ALPHA) * con_loss)
    return (total, ce_loss, con_loss)


# revision 16
# speedup vs baseline: 1.3876x; 1.3876x over previous
"""CLUES loss (focal CE + supervised contrastive) on 8 Trainium2 NeuronCores.

v2: circulant-triangle sharding.  The N x N sim matrix is covered once per
unordered pair: row-tile a (128 rows) covers col-tiles a..a+32 (mod 64).
Core k owns row-tiles {k, 8+k, ..., 56+k}; the host feeds each core its
row-rotated copy of the data so one static program serves all cores.

Per core: fp8 DoubleRow matmuls produce sim psum chunks; ACT applies
exp(k*psum) in place with accum_out row-sums; ap=1 transposed matmuls on the
PE produce per-column sums (diag tile excluded once).  The host sums the 8
cores' row/col partial-S vectors, takes the log, and assembles the loss --
the O(N^2) and O(N*D) work all happens on device; the host does O(N)
unsharding only.

Normalization: row norms from fp8 bn_stats (split DVE/Pool); rb = 8/||x||
broadcast across partitions via a DRAM round trip; the transposed fp8 copy
(host-provided, ktile-major) is scaled by rb on DVE to give xn; xn feeds the
Gram, the class-sum matmuls (via oh*rb weights), and the per-row class dots
M8 used for the masked term.
"""

import sys

if '/opt/trn_rl_repo' not in sys.path:
    sys.path.insert(0, '/opt/trn_rl_repo')

import numpy as np
import ml_dtypes

import concourse.bass as bass
import concourse.mybir as mybir
import concourse.tile as tile
from concourse.vector_clock import ScopedClock
from concourse.bass_utils import run_bass_kernel_spmd

F32 = mybir.dt.float32
BF16 = mybir.dt.bfloat16
FP8 = mybir.dt.float8e4
I32 = mybir.dt.int32
AF = mybir.ActivationFunctionType
ALU = mybir.AluOpType
AX = mybir.AxisListType
DR = mybir.MatmulPerfMode.DoubleRow

FP8NP = ml_dtypes.float8_e4m3
BF16NP = ml_dtypes.bfloat16

N, C, D = 8192, 8, 256
NCORES = 8
T64 = 64                       # col tiles
MT = 8                         # own row tiles (CE side)
GAMMA = 2.0
LS = 0.1
ALPHA = 0.3
TEMP = 0.07
C1 = 8.0                       # xn = x_hat * C1  ->  sim psum = 64*cos
KAPPA = 1.0 / (C1 * C1 * TEMP)  # exp scale: kappa*psum = cos/T
LN_C1 = float(np.log(C1))

STRIPES = [0, 8, 16, 24, 32, 40, 48, 56]
# chunks per stripe: list of (col_tile0, n_tiles); <=12 tiles per chunk
CHUNKS = {
    0:  [(0, 12), (12, 12), (24, 9)],
    8:  [(8, 12), (20, 12), (32, 9)],
    16: [(16, 12), (28, 12), (40, 9)],
    24: [(24, 12), (36, 12), (48, 9)],
    32: [(32, 12), (44, 12), (56, 8)],
    40: [(40, 12), (52, 12), (0, 8)],
    48: [(48, 12), (60, 4), (0, 12), (12, 4)],
    56: [(56, 8), (0, 12), (12, 12)],
}
NCHMAX = 4

# colsum bookkeeping: col tile -> ordered list of (stripe_idx, chunk_idx,
# slice_within_chunk); diag slice (col==stripe) excluded.
_COLTOUCH = {}
for _si, _a in enumerate(STRIPES):
    for _ci, (_c0, _nt) in enumerate(CHUNKS[_a]):
        for _s in range(_nt):
            _c = _c0 + _s
            if _c == _a:
                continue
            _COLTOUCH.setdefault(_c, []).append((_si, _ci, _s))


# ---------------------------------------------------------------------------
# walrus in this container only accepts ONE semaphore wait per instruction,
# while Tile freely attaches several.  Patch 1 fixes the final drain; patch 2
# is a post-pass hoisting extra waits onto same-engine NoOp carriers.
# ---------------------------------------------------------------------------

def _patched_drain_and_barrier(self, tick_clock, wait_clock):
    nc = self.nc
    carrier = nc.sync.nop(nofuse=True, hint="drain_wait_carrier")
    wait_clock.add_sem_waits(carrier.ins, ScopedClock({None: tick_clock.global_clock}))
    si = carrier.ins.sync_info
    waits = list(si.on_wait or []) if si is not None else []
    if len(waits) > 1:
        carrier.ins.sync_info = mybir.SyncInfo(
            on_wait=waits[:1], on_update=list(si.on_update or []))
        for w in waits[1:]:
            n2 = nc.sync.nop(nofuse=True, hint="drain_wait_carrier")
            n2.ins.sync_info = mybir.SyncInfo(on_wait=[w], on_update=[])
    nc.sync.drain()
    nc.all_engine_barrier()
    popped = nc._tile_sem_poison_stack.pop()
    assert popped is self._sem_poison
    nc.clear_and_free_semaphores(list(self.sems.allocated().values()))
    nc.all_engine_barrier()


tile.TileContext._drain_and_barrier = _patched_drain_and_barrier


def _split_multi_waits(nc):
    """One sem wait per instruction: move extras to NoOp carriers just before."""
    n_split = 0
    for f in nc.m.functions:
        for bb in f.blocks:
            new = []
            for inst in bb.instructions:
                si = inst.sync_info
                waits = list(si.on_wait or []) if si is not None else []
                if len(waits) > 1:
                    for w in waits[:-1]:
                        nop = mybir.InstNoOp(
                            name=f"{inst.name}-wsplit{n_split}",
                            engine=inst.engine,
                            bass_nofuse=True,
                            sync_info=mybir.SyncInfo(on_wait=[w], on_update=[]),
                        )
                        n_split += 1
                        new.append(nop)
                    inst.sync_info = mybir.SyncInfo(
                        on_wait=[waits[-1]], on_update=list(si.on_update or []))
                new.append(inst)
            bb.instructions[:] = new


# ---------------------------------------------------------------------------
# kernel build
# ---------------------------------------------------------------------------

def _build():
    nc = bass.Bass()

    xT_d = nc.dram_tensor("xT", [128, 2, N], FP8, kind="ExternalInput")
    xrm_d = nc.dram_tensor("xrm", [N, D], FP8, kind="ExternalInput")
    tpo_d = nc.dram_tensor("tpo", [N], F32, kind="ExternalInput")   # rotated
    lg_d = nc.dram_tensor("lg", [N // 8, C], F32, kind="ExternalInput")
    to_d = nc.dram_tensor("to", [N // 8], F32, kind="ExternalInput")

    sacc_d = nc.dram_tensor("sacc", [128, 8, NCHMAX], F32, kind="ExternalOutput")
    csum_d = nc.dram_tensor("csum", [128, T64], F32, kind="ExternalOutput")
    m8_d = nc.dram_tensor("m8", [128, T64, C], F32, kind="ExternalOutput")
    cnt_d = nc.dram_tensor("cnt", [C, 1], F32, kind="ExternalOutput")
    focal_d = nc.dram_tensor("focal", [128, MT], F32, kind="ExternalOutput")

    r_scr = nc.dram_tensor("r_scr", [N], BF16, kind="Internal")
    cs_scr = nc.dram_tensor("cs_scr", [C, D], BF16, kind="Internal")

    with tile.TileContext(nc) as tc:
        with (
            tc.tile_pool(name="big", bufs=1) as big,
            tc.tile_pool(name="work", bufs=3) as work,
            tc.tile_pool(name="ebfp", bufs=2) as ebfp,
            tc.tile_pool(name="psb", bufs=2, space="PSUM") as psb,
            tc.tile_pool(name="psc", bufs=1, space="PSUM") as psc,
            tc.tile_pool(name="pss", bufs=1, space="PSUM") as pss,
        ):
            # ============ phase 0: loads + ACT warm =========================
            warm = big.tile([128, 1], F32, tag="warm")
            nc.vector.memset(warm, 0.0)
            nc.scalar.activation(out=warm, in_=warm, func=AF.Exp)

            xT8 = big.tile([128, 2, N], FP8, tag="xT8")
            x8rm = big.tile([128, 64, D], FP8, tag="x8rm")
            xrm_ap = xrm_d.ap().rearrange("(t p) d -> p t d", p=128)
            for g in range(8):
                nc.sync.dma_start(out=x8rm[:, g * 8:(g + 1) * 8, :],
                                  in_=xrm_ap[:, g * 8:(g + 1) * 8, :])
                nc.sync.dma_start(out=xT8[:, :, g * 1024:(g + 1) * 1024],
                                  in_=xT_d.ap()[:, :, g * 1024:(g + 1) * 1024])

            lg = big.tile([128, MT, C], F32, tag="lg")
            nc.scalar.dma_start(out=lg, in_=lg_d.ap().rearrange("(t p) c -> p t c", p=128))
            to_pi = big.tile([128, MT], F32, tag="to_pi")
            nc.scalar.dma_start(out=to_pi, in_=to_d.ap().rearrange("(t p) -> p t", p=128))
            t_po = big.tile([128, T64], F32, tag="t_po")
            nc.scalar.dma_start(out=t_po, in_=tpo_d.ap().rearrange("(t p) -> p t", p=128))

            from concourse.masks import make_identity
            ident16 = big.tile([128, 128], BF16, tag="ident16")
            make_identity(nc, ident16)

            # iotas for one-hots
            iota8_i = big.tile([128, C], I32, tag="iota8i")
            nc.gpsimd.iota(iota8_i, pattern=[[1, C]], base=0, channel_multiplier=0)
            iota8 = big.tile([128, C], F32, tag="iota8")
            nc.vector.tensor_copy(out=iota8, in_=iota8_i)

            # ============ phase 1: focal CE (fills ACT early) ===============
            oh_own = big.tile([128, MT, C], F32, tag="oh_own")
            nc.vector.tensor_tensor(
                out=oh_own,
                in0=to_pi.to_broadcast([128, MT, C]),
                in1=bass.AP(tensor=iota8.tensor, offset=iota8.offset,
                            ap=[iota8.ap[0], [0, MT], iota8.ap[1]]),
                op=ALU.is_equal)
            focal = big.tile([128, MT], F32, tag="focal")
            mx = big.tile([128, MT], F32, tag="mx")
            nc.vector.reduce_max(out=mx, in_=lg, axis=AX.X)
            nmx = big.tile([128, MT], F32, tag="nmx")
            nc.vector.tensor_scalar(out=nmx, in0=mx, scalar1=-1.0, scalar2=None,
                                    op0=ALU.mult, op1=ALU.bypass)
            sumexp = big.tile([128, MT], F32, tag="sumexp")
            for m in range(MT):
                esc = work.tile([128, C], F32, tag="esc")
                nc.scalar.activation(out=esc, in_=lg[:, m, :], func=AF.Exp,
                                     bias=nmx[:, m:m + 1], scale=1.0,
                                     accum_out=sumexp[:, m:m + 1])
            logZ = big.tile([128, MT], F32, tag="logZ")
            nc.scalar.activation(out=logZ, in_=sumexp, func=AF.Ln)
            nc.vector.tensor_tensor(out=logZ, in0=logZ, in1=mx, op=ALU.add)
            xt = big.tile([128, MT], F32, tag="xt")
            ohl = work.tile([128, MT, C], F32, tag="ohl")
            nc.vector.tensor_tensor(out=ohl, in0=lg, in1=oh_own, op=ALU.mult)
            nc.vector.reduce_sum(out=xt, in_=ohl, axis=AX.X)
            sx = big.tile([128, MT], F32, tag="sx")
            nc.vector.reduce_sum(out=sx, in_=lg, axis=AX.X)
            ce = big.tile([128, MT], F32, tag="ce")
            u1 = work.tile([128, MT], F32, tag="u1")
            nc.vector.tensor_scalar(out=u1, in0=xt, scalar1=1.0 - LS, scalar2=None,
                                    op0=ALU.mult, op1=ALU.bypass)
            u2 = work.tile([128, MT], F32, tag="u2")
            nc.vector.tensor_scalar(out=u2, in0=sx, scalar1=LS / C, scalar2=None,
                                    op0=ALU.mult, op1=ALU.bypass)
            nc.vector.tensor_tensor(out=u1, in0=u1, in1=u2, op=ALU.add)
            nc.vector.tensor_tensor(out=ce, in0=logZ, in1=u1, op=ALU.subtract)
            pt_t = work.tile([128, MT], F32, tag="pt")
            nc.scalar.activation(out=pt_t, in_=ce, func=AF.Exp, scale=-1.0)
            nc.vector.tensor_scalar(out=pt_t, in0=pt_t, scalar1=-1.0, scalar2=1.0,
                                    op0=ALU.mult, op1=ALU.add)
            nc.vector.tensor_tensor(out=focal, in0=pt_t, in1=pt_t, op=ALU.mult)
            nc.vector.tensor_tensor(out=focal, in0=focal, in1=ce, op=ALU.mult)
            nc.sync.dma_start(out=focal_d.ap(), in_=focal)

            # one-hot (p-outer) for class sums / counts
            oh_po = big.tile([128, T64, C], BF16, tag="oh_po")
            nc.vector.tensor_tensor(
                out=oh_po,
                in0=t_po.to_broadcast([128, T64, C]),
                in1=bass.AP(tensor=iota8.tensor, offset=iota8.offset,
                            ap=[iota8.ap[0], [0, T64], iota8.ap[1]]),
                op=ALU.is_equal)

            # ============ phase 2: stats -> rb -> xn (pipelined by group) ===
            sumsq = big.tile([128, T64], F32, tag="sumsq")
            lnc1_t = big.tile([128, 1], F32, tag="lnc1")
            nc.vector.memset(lnc1_t, LN_C1)
            rb_po = big.tile([128, T64], BF16, tag="rb_po")
            rb16bc = big.tile([128, N], BF16, tag="rb16bc")
            xn = big.tile([128, 2, N], FP8, tag="xn")
            r_po_ap = r_scr.ap().rearrange("(t p) -> p t", p=128)

            for g in range(8):
                t0 = g * 8
                for t in range(t0, t0 + 8):
                    sqp = work.tile([128, D], BF16, tag="sqp")
                    nc.gpsimd.tensor_tensor(out=sqp, in0=x8rm[:, t, :],
                                            in1=x8rm[:, t, :], op=ALU.mult)
                    nc.vector.reduce_sum(out=sumsq[:, t:t + 1], in_=sqp,
                                         axis=AX.X)
                # rb = C1 / sqrt(sumsq) = exp(-0.5*ln(sumsq) + ln C1)
                lns = work.tile([128, 8], F32, tag="lns")
                nc.scalar.activation(out=lns, in_=sumsq[:, t0:t0 + 8], func=AF.Ln)
                nc.scalar.activation(out=rb_po[:, t0:t0 + 8], in_=lns,
                                     func=AF.Exp, scale=-0.5, bias=lnc1_t[:, 0:1])
                nc.scalar.dma_start(out=r_po_ap[:, t0:t0 + 8],
                                    in_=rb_po[:, t0:t0 + 8])
                # broadcast rb over partitions for cols of this group
                j0 = t0 * 128
                nc.scalar.dma_start(
                    out=rb16bc[:, j0:j0 + 1024],
                    in_=bass.AP(tensor=r_scr.ap().tensor, offset=j0,
                                ap=[[0, 128], [1, 1024]]))
                for kt in range(2):
                    nc.vector.tensor_tensor(out=xn[:, kt, j0:j0 + 1024],
                                            in0=xT8[:, kt, j0:j0 + 1024],
                                            in1=rb16bc[:, j0:j0 + 1024],
                                            op=ALU.mult)

            # ============ phase 3: class sums + counts (PE, off critical) ===
            ohr = big.tile([128, T64, C], FP8, tag="ohr")
            nc.vector.tensor_tensor(
                out=ohr, in0=oh_po,
                in1=bass.AP(tensor=rb_po.tensor, offset=rb_po.offset,
                            ap=[rb_po.ap[0], rb_po.ap[1], [0, C]]),
                op=ALU.mult)
            ones_bf = big.tile([128, 1], BF16, tag="ones_bf")
            nc.vector.memset(ones_bf, 1.0)
            cs_ps = pss.tile([C, D + 1], F32, tag="cs")
            for t in range(T64):
                nc.tensor.matmul(cs_ps[:, 0:D], ohr[:, t, :], x8rm[:, t, :],
                                 start=(t == 0), stop=(t == T64 - 1),
                                 skip_group_check=True)
                nc.tensor.matmul(cs_ps[:, D:D + 1], oh_po[:, t, :], ones_bf,
                                 start=(t == 0), stop=(t == T64 - 1),
                                 skip_group_check=True)
            cnt_sb = big.tile([C, 1], F32, tag="cnt_sb")
            nc.vector.tensor_copy(out=cnt_sb, in_=cs_ps[:, D:D + 1])
            nc.sync.dma_start(out=cnt_d.ap(), in_=cnt_sb)
            cs16 = big.tile([C, D], BF16, tag="cs16")
            nc.vector.tensor_copy(out=cs16, in_=cs_ps[:, 0:D])
            # transpose CS to ktile-major fp8 [128, 2, C] via tiny DRAM trip
            nc.sync.dma_start(out=cs_scr.ap(), in_=cs16)
            cs8t16 = big.tile([128, 2, C], BF16, tag="cs8t16")
            for kt in range(2):
                nc.sync.dma_start(out=cs8t16[:, kt, :], in_=bass.AP(
                    tensor=cs_scr.ap().tensor, offset=kt * 128,
                    ap=[[1, 128], [D, C]]))
            cs8t = big.tile([128, 2, C], FP8, tag="cs8t")
            nc.vector.tensor_copy(out=cs8t, in_=cs8t16)

            # ============ phase 4: stripes: gram + exp + colsums ============
            # software-pipelined: chunk i+1's matmuls are emitted before
            # chunk i's exp-dependent colsums so the PE never parks behind ACT
            sacc = big.tile([128, 8, NCHMAX], F32, tag="sacc")
            nc.vector.memset(sacc, 0.0)
            col_ps = psc.tile([128, T64], F32, tag="colps")
            touched = {c: 0 for c in _COLTOUCH}

            allchunks = [(si, a, ci, c0, nt)
                         for si, a in enumerate(STRIPES)
                         for ci, (c0, nt) in enumerate(CHUNKS[a])]

            def emit_mm(i):
                si, a, ci, c0, nt = allchunks[i]
                csz = nt * 128
                pt = psb.tile([128, 1536], F32, tag="bigps")
                for n0 in range(0, csz, 512):
                    w = min(512, csz - n0)
                    nc.tensor.matmul(
                        pt[:, n0:n0 + w], xn[:, :, a * 128:(a + 1) * 128],
                        xn[:, :, (c0 * 128) + n0:(c0 * 128) + n0 + w],
                        start=True, stop=True, perf_mode=DR)
                return pt

            def emit_exp(i, pt):
                si, a, ci, c0, nt = allchunks[i]
                csz = nt * 128
                ebf = ebfp.tile([128, 1536], BF16, tag="ebf")
                nc.scalar.activation(out=ebf[:, 0:csz], in_=pt[:, 0:csz],
                                     func=AF.Exp, scale=KAPPA,
                                     accum_out=sacc[:, si, ci:ci + 1])
                return ebf

            def emit_cols(i, ebf):
                si, a, ci, c0, nt = allchunks[i]
                for s in range(nt):
                    c = c0 + s
                    if c == a:
                        continue
                    touched[c] += 1
                    nc.tensor.matmul(col_ps[:, c:c + 1],
                                     ebf[:, s * 128:(s + 1) * 128], ones_bf,
                                     start=(touched[c] == 1),
                                     stop=(touched[c] == len(_COLTOUCH[c])),
                                     skip_group_check=True)

            prev = None
            for i in range(len(allchunks)):
                pt = emit_mm(i)
                if prev is not None:
                    emit_cols(prev[0], prev[1])
                ebf = emit_exp(i, pt)
                prev = (i, ebf)
            emit_cols(prev[0], prev[1])

            # M8[row, c] = xn_row . CS8T (DoubleRow; cheap tail on the PE)
            m8_full = psb.tile([128, 1536], F32, tag="bigps")
            m8_ps = m8_full[:, 0:T64 * C]
            for t in range(T64):
                nc.tensor.matmul(m8_ps[:, t * C:(t + 1) * C],
                                 xn[:, :, t * 128:(t + 1) * 128], cs8t,
                                 start=True, stop=True, perf_mode=DR,
                                 skip_group_check=True)
            m8_sb = big.tile([128, T64 * C], F32, tag="m8sb")
            nc.vector.tensor_copy(out=m8_sb, in_=m8_ps)
            nc.sync.dma_start(out=m8_d.ap().rearrange("p t c -> p (t c)"),
                              in_=m8_sb)

            # ============ phase 5: outputs ==================================
            col_sb = big.tile([128, T64], F32, tag="col_sb")
            nc.vector.tensor_copy(out=col_sb, in_=col_ps)
            nc.sync.dma_start(out=csum_d.ap(), in_=col_sb)
            nc.sync.dma_start(out=sacc_d.ap(), in_=sacc)

    _split_multi_waits(nc)
    return nc


_NC = None
LAST_RESULTS = None
RUN_KWARGS = {}


def _get_nc():
    global _NC
    if _NC is None:
        _NC = _build()
    return _NC


def kernel(logits, embeddings, targets):
    logits = np.ascontiguousarray(np.asarray(logits), dtype=np.float32)
    embeddings = np.ascontiguousarray(np.asarray(embeddings), dtype=np.float32)
    targets_np = np.asarray(targets)
    tf32 = targets_np.astype(np.float32)

    x8 = embeddings.astype(FP8NP)                      # [N, D] fp8
    nc = _get_nc()
    in_maps = []
    for k in range(NCORES):
        roll = np.roll(x8, -k * 128, axis=0)
        xT8 = np.ascontiguousarray(roll.reshape(N, 2, 128).transpose(2, 1, 0))
        sl = slice(k * (N // 8), (k + 1) * (N // 8))
        in_maps.append({
            "xT": xT8,
            "xrm": np.ascontiguousarray(roll),
            "tpo": np.ascontiguousarray(np.roll(tf32, -k * 128)),
            "lg": np.ascontiguousarray(logits[sl]),
            "to": np.ascontiguousarray(tf32[sl]),
        })
    res = run_bass_kernel_spmd(nc, in_maps, core_ids=list(range(NCORES)), **RUN_KWARGS)
    global LAST_RESULTS
    LAST_RESULTS = res

    # ---- host unshard: O(N) assembly --------------------------------------
    S = np.zeros(N, np.float64)
    fsum = 0.0
    for k in range(NCORES):
        r = res.results[k]
        sacc = np.asarray(r["sacc"], np.float64)       # [128, 8, 4]
        for si, a in enumerate(STRIPES):
            b = (a + k) % T64
            S[b * 128:(b + 1) * 128] += sacc[:, si, :].sum(axis=1)
        csum = np.asarray(r["csum"], np.float64)       # [128, 64]
        for c in range(T64):
            b = (c + k) % T64
            S[b * 128:(b + 1) * 128] += csum[:, c]
        fsum += float(np.asarray(r["focal"], np.float64).sum())

    r0 = res.results[0]
    m8 = np.asarray(r0["m8"], np.float64)              # [128, 64, 8] p-inner
    m8r = m8.transpose(1, 0, 2).reshape(N, C)          # row = t*128+p
    cnt = np.asarray(r0["cnt"], np.float64).reshape(C)

    tgt = targets_np.astype(np.int64)
    P = cnt[tgt] - 1.0
    npos = np.maximum(P, 1.0)
    masked = m8r[np.arange(N), tgt] * KAPPA - 1.0 / TEMP
    con_i = (P * np.log(S + 1e-8) - masked) / npos
    con_loss = np.float32(con_i.mean())
    ce_loss = np.float32(fsum / N)
    total = np.float32(ce_loss + np.float32(ALPHA) * con_loss)
    return (total, ce_loss, con_loss)


# revision 19
# speedup vs baseline: 1.5604x; 1.1245x over previous
"""CLUES loss (focal CE + supervised contrastive) on 8 Trainium2 NeuronCores.

v2: circulant-triangle sharding.  The N x N sim matrix is covered once per
unordered pair: row-tile a (128 rows) covers col-tiles a..a+32 (mod 64).
Core k owns row-tiles {k, 8+k, ..., 56+k}; the host feeds each core its
row-rotated copy of the data so one static program serves all cores.

Per core: fp8 DoubleRow matmuls produce sim psum chunks; ACT applies
exp(k*psum) in place with accum_out row-sums; ap=1 transposed matmuls on the
PE produce per-column sums (diag tile excluded once).  The host sums the 8
cores' row/col partial-S vectors, takes the log, and assembles the loss --
the O(N^2) and O(N*D) work all happens on device; the host does O(N)
unsharding only.

Normalization: row norms from fp8 bn_stats (split DVE/Pool); rb = 8/||x||
broadcast across partitions via a DRAM round trip; the transposed fp8 copy
(host-provided, ktile-major) is scaled by rb on DVE to give xn; xn feeds the
Gram, the class-sum matmuls (via oh*rb weights), and the per-row class dots
M8 used for the masked term.
"""

import sys

if '/opt/trn_rl_repo' not in sys.path:
    sys.path.insert(0, '/opt/trn_rl_repo')

import numpy as np
import ml_dtypes

import concourse.bass as bass
import concourse.mybir as mybir
import concourse.tile as tile
from concourse.vector_clock import ScopedClock
from concourse.bass_utils import run_bass_kernel_spmd

F32 = mybir.dt.float32
BF16 = mybir.dt.bfloat16
FP8 = mybir.dt.float8e4
I32 = mybir.dt.int32
AF = mybir.ActivationFunctionType
ALU = mybir.AluOpType
AX = mybir.AxisListType
DR = mybir.MatmulPerfMode.DoubleRow

FP8NP = ml_dtypes.float8_e4m3
BF16NP = ml_dtypes.bfloat16

N, C, D = 8192, 8, 256
NCORES = 8
T64 = 64                       # col tiles
MT = 8                         # own row tiles (CE side)
GAMMA = 2.0
LS = 0.1
ALPHA = 0.3
TEMP = 0.07
C1 = 8.0                       # xn = x_hat * C1  ->  sim psum = 64*cos
KAPPA = 1.0 / (C1 * C1 * TEMP)  # exp scale: kappa*psum = cos/T
LN_C1 = float(np.log(C1))

STRIPES = [0, 8, 16, 24, 32, 40, 48, 56]
# chunks per stripe: list of (col_tile0, n_tiles); <=12 tiles per chunk
CHUNKS = {
    0:  [(0, 12), (12, 12), (24, 9)],
    8:  [(8, 12), (20, 12), (32, 9)],
    16: [(16, 12), (28, 12), (40, 9)],
    24: [(24, 12), (36, 12), (48, 9)],
    32: [(32, 12), (44, 12), (56, 8)],
    40: [(40, 12), (52, 12), (0, 8)],
    48: [(48, 12), (60, 4), (0, 12), (12, 4)],
    56: [(56, 8), (0, 12), (12, 12)],
}
NCHMAX = 4

# colsum bookkeeping: col tile -> ordered list of (stripe_idx, chunk_idx,
# slice_within_chunk); diag slice (col==stripe) excluded.
_COLTOUCH = {}
for _si, _a in enumerate(STRIPES):
    for _ci, (_c0, _nt) in enumerate(CHUNKS[_a]):
        for _s in range(_nt):
            _c = _c0 + _s
            if _c == _a:
                continue
            _COLTOUCH.setdefault(_c, []).append((_si, _ci, _s))


# ---------------------------------------------------------------------------
# walrus in this container only accepts ONE semaphore wait per instruction,
# while Tile freely attaches several.  Patch 1 fixes the final drain; patch 2
# is a post-pass hoisting extra waits onto same-engine NoOp carriers.
# ---------------------------------------------------------------------------

def _patched_drain_and_barrier(self, tick_clock, wait_clock):
    nc = self.nc
    carrier = nc.sync.nop(nofuse=True, hint="drain_wait_carrier")
    wait_clock.add_sem_waits(carrier.ins, ScopedClock({None: tick_clock.global_clock}))
    si = carrier.ins.sync_info
    waits = list(si.on_wait or []) if si is not None else []
    if len(waits) > 1:
        carrier.ins.sync_info = mybir.SyncInfo(
            on_wait=waits[:1], on_update=list(si.on_update or []))
        for w in waits[1:]:
            n2 = nc.sync.nop(nofuse=True, hint="drain_wait_carrier")
            n2.ins.sync_info = mybir.SyncInfo(on_wait=[w], on_update=[])
    nc.sync.drain()
    nc.all_engine_barrier()
    popped = nc._tile_sem_poison_stack.pop()
    assert popped is self._sem_poison
    nc.clear_and_free_semaphores(list(self.sems.allocated().values()))
    nc.all_engine_barrier()


tile.TileContext._drain_and_barrier = _patched_drain_and_barrier


def _split_multi_waits(nc):
    """One sem wait per instruction: move extras to NoOp carriers just before."""
    n_split = 0
    for f in nc.m.functions:
        for bb in f.blocks:
            new = []
            for inst in bb.instructions:
                si = inst.sync_info
                waits = list(si.on_wait or []) if si is not None else []
                if len(waits) > 1:
                    for w in waits[:-1]:
                        nop = mybir.InstNoOp(
                            name=f"{inst.name}-wsplit{n_split}",
                            engine=inst.engine,
                            bass_nofuse=True,
                            sync_info=mybir.SyncInfo(on_wait=[w], on_update=[]),
                        )
                        n_split += 1
                        new.append(nop)
                    inst.sync_info = mybir.SyncInfo(
                        on_wait=[waits[-1]], on_update=list(si.on_update or []))
                new.append(inst)
            bb.instructions[:] = new


# ---------------------------------------------------------------------------
# kernel build
# ---------------------------------------------------------------------------

def _build():
    nc = bass.Bass()

    xT_d = nc.dram_tensor("xT", [128, 2, N], FP8, kind="ExternalInput")
    xrm_d = nc.dram_tensor("xrm", [N, D], FP8, kind="ExternalInput")
    tpo_d = nc.dram_tensor("tpo", [N], F32, kind="ExternalInput")   # rotated
    lg_d = nc.dram_tensor("lg", [N // 8, C], F32, kind="ExternalInput")
    to_d = nc.dram_tensor("to", [N // 8], F32, kind="ExternalInput")

    sacc_d = nc.dram_tensor("sacc", [128, 8, NCHMAX], F32, kind="ExternalOutput")
    csum_d = nc.dram_tensor("csum", [128, T64], F32, kind="ExternalOutput")
    m8_d = nc.dram_tensor("m8", [128, T64, C], F32, kind="ExternalOutput")
    cnt_d = nc.dram_tensor("cnt", [C, 1], F32, kind="ExternalOutput")
    focal_d = nc.dram_tensor("focal", [128, MT], F32, kind="ExternalOutput")

    r_scr = nc.dram_tensor("r_scr", [N], BF16, kind="Internal")
    cs_scr = nc.dram_tensor("cs_scr", [C, D], BF16, kind="Internal")

    with tile.TileContext(nc) as tc:
        with (
            tc.tile_pool(name="big", bufs=1) as big,
            tc.tile_pool(name="work", bufs=3) as work,
            tc.tile_pool(name="ebfp", bufs=2) as ebfp,
            tc.tile_pool(name="psb", bufs=2, space="PSUM") as psb,
            tc.tile_pool(name="psc", bufs=1, space="PSUM") as psc,
            tc.tile_pool(name="pss", bufs=1, space="PSUM") as pss,
        ):
            # ============ phase 0: loads + ACT warm =========================
            warm = big.tile([128, 1], F32, tag="warm")
            nc.vector.memset(warm, 0.0)
            nc.scalar.activation(out=warm, in_=warm, func=AF.Exp)

            xT8 = big.tile([128, 2, N], FP8, tag="xT8")
            x8rm = big.tile([128, 64, D], FP8, tag="x8rm")
            xrm_ap = xrm_d.ap().rearrange("(t p) d -> p t d", p=128)
            for g in range(8):
                nc.sync.dma_start(out=x8rm[:, g * 8:(g + 1) * 8, :],
                                  in_=xrm_ap[:, g * 8:(g + 1) * 8, :])
                nc.sync.dma_start(out=xT8[:, :, g * 1024:(g + 1) * 1024],
                                  in_=xT_d.ap()[:, :, g * 1024:(g + 1) * 1024])

            lg = big.tile([128, MT, C], F32, tag="lg")
            nc.scalar.dma_start(out=lg, in_=lg_d.ap().rearrange("(t p) c -> p t c", p=128))
            to_pi = big.tile([128, MT], F32, tag="to_pi")
            nc.scalar.dma_start(out=to_pi, in_=to_d.ap().rearrange("(t p) -> p t", p=128))
            t_po = big.tile([128, T64], F32, tag="t_po")
            nc.scalar.dma_start(out=t_po, in_=tpo_d.ap().rearrange("(t p) -> p t", p=128))

            from concourse.masks import make_identity
            ident16 = big.tile([128, 128], BF16, tag="ident16")
            make_identity(nc, ident16)

            # iotas for one-hots
            iota8_i = big.tile([128, C], I32, tag="iota8i")
            nc.gpsimd.iota(iota8_i, pattern=[[1, C]], base=0, channel_multiplier=0)
            iota8 = big.tile([128, C], F32, tag="iota8")
            nc.vector.tensor_copy(out=iota8, in_=iota8_i)

            # ============ phase 1: focal CE (fills ACT early) ===============
            oh_own = big.tile([128, MT, C], F32, tag="oh_own")
            nc.vector.tensor_tensor(
                out=oh_own,
                in0=to_pi.to_broadcast([128, MT, C]),
                in1=bass.AP(tensor=iota8.tensor, offset=iota8.offset,
                            ap=[iota8.ap[0], [0, MT], iota8.ap[1]]),
                op=ALU.is_equal)
            focal = big.tile([128, MT], F32, tag="focal")
            mx = big.tile([128, MT], F32, tag="mx")
            nc.vector.reduce_max(out=mx, in_=lg, axis=AX.X)
            nmx = big.tile([128, MT], F32, tag="nmx")
            nc.vector.tensor_scalar(out=nmx, in0=mx, scalar1=-1.0, scalar2=None,
                                    op0=ALU.mult, op1=ALU.bypass)
            sumexp = big.tile([128, MT], F32, tag="sumexp")
            for m in range(MT):
                esc = work.tile([128, C], F32, tag="esc")
                nc.scalar.activation(out=esc, in_=lg[:, m, :], func=AF.Exp,
                                     bias=nmx[:, m:m + 1], scale=1.0,
                                     accum_out=sumexp[:, m:m + 1])
            logZ = big.tile([128, MT], F32, tag="logZ")
            nc.scalar.activation(out=logZ, in_=sumexp, func=AF.Ln)
            nc.vector.tensor_tensor(out=logZ, in0=logZ, in1=mx, op=ALU.add)
            xt = big.tile([128, MT], F32, tag="xt")
            ohl = work.tile([128, MT, C], F32, tag="ohl")
            nc.vector.tensor_tensor(out=ohl, in0=lg, in1=oh_own, op=ALU.mult)
            nc.vector.reduce_sum(out=xt, in_=ohl, axis=AX.X)
            sx = big.tile([128, MT], F32, tag="sx")
            nc.vector.reduce_sum(out=sx, in_=lg, axis=AX.X)
            ce = big.tile([128, MT], F32, tag="ce")
            u1 = work.tile([128, MT], F32, tag="u1")
            nc.vector.tensor_scalar(out=u1, in0=xt, scalar1=1.0 - LS, scalar2=None,
                                    op0=ALU.mult, op1=ALU.bypass)
            u2 = work.tile([128, MT], F32, tag="u2")
            nc.vector.tensor_scalar(out=u2, in0=sx, scalar1=LS / C, scalar2=None,
                                    op0=ALU.mult, op1=ALU.bypass)
            nc.vector.tensor_tensor(out=u1, in0=u1, in1=u2, op=ALU.add)
            nc.vector.tensor_tensor(out=ce, in0=logZ, in1=u1, op=ALU.subtract)
            pt_t = work.tile([128, MT], F32, tag="pt")
            nc.scalar.activation(out=pt_t, in_=ce, func=AF.Exp, scale=-1.0)
            nc.vector.tensor_scalar(out=pt_t, in0=pt_t, scalar1=-1.0, scalar2=1.0,
                                    op0=ALU.mult, op1=ALU.add)
            nc.vector.tensor_tensor(out=focal, in0=pt_t, in1=pt_t, op=ALU.mult)
            nc.vector.tensor_tensor(out=focal, in0=focal, in1=ce, op=ALU.mult)
            nc.sync.dma_start(out=focal_d.ap(), in_=focal)

            # one-hot (p-outer) for class sums / counts
            oh_po = big.tile([128, T64, C], BF16, tag="oh_po")
            nc.vector.tensor_tensor(
                out=oh_po,
                in0=t_po.to_broadcast([128, T64, C]),
                in1=bass.AP(tensor=iota8.tensor, offset=iota8.offset,
                            ap=[iota8.ap[0], [0, T64], iota8.ap[1]]),
                op=ALU.is_equal)

            # ============ phase 2: stats -> rb -> xn (pipelined by group) ===
            sumsq = big.tile([128, T64], F32, tag="sumsq")
            lnc1_t = big.tile([128, 1], F32, tag="lnc1")
            nc.vector.memset(lnc1_t, LN_C1)
            rb_po = big.tile([128, T64], BF16, tag="rb_po")
            rb16bc = big.tile([128, N], BF16, tag="rb16bc")
            xn = big.tile([128, 2, N], FP8, tag="xn")
            r_po_ap = r_scr.ap().rearrange("(t p) -> p t", p=128)

            for g in range(8):
                t0 = g * 8
                for t in range(t0, t0 + 8):
                    bn = work.tile([128, 6], F32, tag="bn")
                    nc.vector.bn_stats(out=bn, in_=x8rm[:, t, :])
                    agg = work.tile([128, 2], F32, tag="agg")
                    nc.vector.bn_aggr(out=agg, in_=bn)
                    m2 = work.tile([128, 1], F32, tag="m2s")
                    nc.vector.tensor_tensor(out=m2, in0=agg[:, 0:1],
                                            in1=agg[:, 0:1], op=ALU.mult)
                    nc.vector.tensor_tensor(out=m2, in0=m2, in1=agg[:, 1:2],
                                            op=ALU.add)
                    nc.vector.tensor_scalar(out=sumsq[:, t:t + 1], in0=m2,
                                            scalar1=float(D), scalar2=None,
                                            op0=ALU.mult, op1=ALU.bypass)
                # rb = C1 / sqrt(sumsq) = exp(-0.5*ln(sumsq) + ln C1)
                lns = work.tile([128, 8], F32, tag="lns")
                nc.scalar.activation(out=lns, in_=sumsq[:, t0:t0 + 8], func=AF.Ln)
                nc.scalar.activation(out=rb_po[:, t0:t0 + 8], in_=lns,
                                     func=AF.Exp, scale=-0.5, bias=lnc1_t[:, 0:1])
                nc.scalar.dma_start(out=r_po_ap[:, t0:t0 + 8],
                                    in_=rb_po[:, t0:t0 + 8])
                # broadcast rb over partitions for cols of this group
                j0 = t0 * 128
                nc.scalar.dma_start(
                    out=rb16bc[:, j0:j0 + 1024],
                    in_=bass.AP(tensor=r_scr.ap().tensor, offset=j0,
                                ap=[[0, 128], [1, 1024]]))
                for kt in range(2):
                    nc.gpsimd.tensor_tensor(out=xn[:, kt, j0:j0 + 1024],
                                            in0=xT8[:, kt, j0:j0 + 1024],
                                            in1=rb16bc[:, j0:j0 + 1024],
                                            op=ALU.mult)

            # ============ phase 3: class sums + counts (PE, off critical) ===
            ohr = big.tile([128, T64, C], FP8, tag="ohr")
            nc.vector.tensor_tensor(
                out=ohr, in0=oh_po,
                in1=bass.AP(tensor=rb_po.tensor, offset=rb_po.offset,
                            ap=[rb_po.ap[0], rb_po.ap[1], [0, C]]),
                op=ALU.mult)
            ones_bf = big.tile([128, 1], BF16, tag="ones_bf")
            nc.vector.memset(ones_bf, 1.0)
            cs_ps = pss.tile([C, D + 1], F32, tag="cs")
            for t in range(T64):
                nc.tensor.matmul(cs_ps[:, 0:D], ohr[:, t, :], x8rm[:, t, :],
                                 start=(t == 0), stop=(t == T64 - 1),
                                 skip_group_check=True)
                nc.tensor.matmul(cs_ps[:, D:D + 1], oh_po[:, t, :], ones_bf,
                                 start=(t == 0), stop=(t == T64 - 1),
                                 skip_group_check=True)
            cnt_sb = big.tile([C, 1], F32, tag="cnt_sb")
            nc.vector.tensor_copy(out=cnt_sb, in_=cs_ps[:, D:D + 1])
            nc.sync.dma_start(out=cnt_d.ap(), in_=cnt_sb)
            cs16 = big.tile([C, D], BF16, tag="cs16")
            nc.vector.tensor_copy(out=cs16, in_=cs_ps[:, 0:D])
            # transpose CS to ktile-major fp8 [128, 2, C] via tiny DRAM trip
            nc.sync.dma_start(out=cs_scr.ap(), in_=cs16)
            cs8t16 = big.tile([128, 2, C], BF16, tag="cs8t16")
            for kt in range(2):
                nc.sync.dma_start(out=cs8t16[:, kt, :], in_=bass.AP(
                    tensor=cs_scr.ap().tensor, offset=kt * 128,
                    ap=[[1, 128], [D, C]]))
            cs8t = big.tile([128, 2, C], FP8, tag="cs8t")
            nc.vector.tensor_copy(out=cs8t, in_=cs8t16)

            # ============ phase 4: stripes: gram + exp + colsums ============
            # software-pipelined: chunk i+1's matmuls are emitted before
            # chunk i's exp-dependent colsums so the PE never parks behind ACT
            sacc = big.tile([128, 8, NCHMAX], F32, tag="sacc")
            nc.vector.memset(sacc, 0.0)
            col_ps = psc.tile([128, T64], F32, tag="colps")
            touched = {c: 0 for c in _COLTOUCH}

            allchunks = [(si, a, ci, c0, nt)
                         for si, a in enumerate(STRIPES)
                         for ci, (c0, nt) in enumerate(CHUNKS[a])]
            # process chunks in order of the latest column-group they need so
            # early chunks never wait on late normalize groups
            allchunks.sort(key=lambda x: max(x[1] // 8, (x[3] + x[4] - 1) // 8))

            def emit_mm(i):
                si, a, ci, c0, nt = allchunks[i]
                csz = nt * 128
                pt = psb.tile([128, 1536], F32, tag="bigps")
                for n0 in range(0, csz, 512):
                    w = min(512, csz - n0)
                    nc.tensor.matmul(
                        pt[:, n0:n0 + w], xn[:, :, a * 128:(a + 1) * 128],
                        xn[:, :, (c0 * 128) + n0:(c0 * 128) + n0 + w],
                        start=True, stop=True, perf_mode=DR)
                return pt

            def emit_exp(i, pt):
                si, a, ci, c0, nt = allchunks[i]
                csz = nt * 128
                ebf = ebfp.tile([128, 1536], BF16, tag="ebf")
                nc.scalar.activation(out=ebf[:, 0:csz], in_=pt[:, 0:csz],
                                     func=AF.Exp, scale=KAPPA,
                                     accum_out=sacc[:, si, ci:ci + 1])
                return ebf

            def emit_cols(i, ebf):
                si, a, ci, c0, nt = allchunks[i]
                for s in range(nt):
                    c = c0 + s
                    if c == a:
                        continue
                    touched[c] += 1
                    nc.tensor.matmul(col_ps[:, c:c + 1],
                                     ebf[:, s * 128:(s + 1) * 128], ones_bf,
                                     start=(touched[c] == 1),
                                     stop=(touched[c] == len(_COLTOUCH[c])),
                                     skip_group_check=True)

            prev = None
            for i in range(len(allchunks)):
                pt = emit_mm(i)
                if prev is not None:
                    emit_cols(prev[0], prev[1])
                ebf = emit_exp(i, pt)
                prev = (i, ebf)
            emit_cols(prev[0], prev[1])

            # M8[row, c] = xn_row . CS8T (DoubleRow; cheap tail on the PE)
            m8_full = psb.tile([128, 1536], F32, tag="bigps")
            m8_ps = m8_full[:, 0:T64 * C]
            for t in range(T64):
                nc.tensor.matmul(m8_ps[:, t * C:(t + 1) * C],
                                 xn[:, :, t * 128:(t + 1) * 128], cs8t,
                                 start=True, stop=True, perf_mode=DR,
                                 skip_group_check=True)
            m8_sb = big.tile([128, T64 * C], F32, tag="m8sb")
            nc.vector.tensor_copy(out=m8_sb, in_=m8_ps)
            nc.sync.dma_start(out=m8_d.ap().rearrange("p t c -> p (t c)"),
                              in_=m8_sb)

            # ============ phase 5: outputs ==================================
            col_sb = big.tile([128, T64], F32, tag="col_sb")
            nc.vector.tensor_copy(out=col_sb, in_=col_ps)
            nc.sync.dma_start(out=csum_d.ap(), in_=col_sb)
            nc.sync.dma_start(out=sacc_d.ap(), in_=sacc)

    _split_multi_waits(nc)
    return nc


_NC = None
LAST_RESULTS = None
RUN_KWARGS = {}


def _get_nc():
    global _NC
    if _NC is None:
        _NC = _build()
    return _NC


def kernel(logits, embeddings, targets):
    logits = np.ascontiguousarray(np.asarray(logits), dtype=np.float32)
    embeddings = np.ascontiguousarray(np.asarray(embeddings), dtype=np.float32)
    targets_np = np.asarray(targets)
    tf32 = targets_np.astype(np.float32)

    x8 = embeddings.astype(FP8NP)                      # [N, D] fp8
    nc = _get_nc()
    in_maps = []
    for k in range(NCORES):
        roll = np.roll(x8, -k * 128, axis=0)
        xT8 = np.ascontiguousarray(roll.reshape(N, 2, 128).transpose(2, 1, 0))
        sl = slice(k * (N // 8), (k + 1) * (N // 8))
        in_maps.append({
            "xT": xT8,
            "xrm": np.ascontiguousarray(roll),
            "tpo": np.ascontiguousarray(np.roll(tf32, -k * 128)),
            "lg": np.ascontiguousarray(logits[sl]),
            "to": np.ascontiguousarray(tf32[sl]),
        })
    res = run_bass_kernel_spmd(nc, in_maps, core_ids=list(range(NCORES)), **RUN_KWARGS)
    global LAST_RESULTS
    LAST_RESULTS = res

    # ---- host unshard: O(N) assembly --------------------------------------
    S = np.zeros(N, np.float64)
    fsum = 0.0
    for k in range(NCORES):
        r = res.results[k]
        sacc = np.asarray(r["sacc"], np.float64)       # [128, 8, 4]
        for si, a in enumerate(STRIPES):
            b = (a + k) % T64
            S[b * 128:(b + 1) * 128] += sacc[:, si, :].sum(axis=1)
        csum = np.asarray(r["csum"], np.float64)       # [128, 64]
        for c in range(T64):
            b = (c + k) % T64
            S[b * 128:(b + 1) * 128] += csum[:, c]
        fsum += float(np.asarray(r["focal"], np.float64).sum())

    r0 = res.results[0]
    m8 = np.asarray(r0["m8"], np.float64)              # [128, 64, 8] p-inner
    m8r = m8.transpose(1, 0, 2).reshape(N, C)          # row = t*128+p
    cnt = np.asarray(r0["cnt"], np.float64).reshape(C)

    tgt = targets_np.astype(np.int64)
    P = cnt[tgt] - 1.0
    npos = np.maximum(P, 1.0)
    masked = m8r[np.arange(N), tgt] * KAPPA - 1.0 / TEMP
    con_i = (P * np.log(S + 1e-8) - masked) / npos
    con_loss = np.float32(con_i.mean())
    ce_loss = np.float32(fsum / N)
    total = np.float32(ce_loss + np.float32(ALPHA) * con_loss)
    return (total, ce_loss, con_loss)
